# revision 18
# baseline (speedup 1.0000x reference)
"""Trainium2 Bass kernel for the categorical-loss nn.Module.

Computation (matching the single-device jax reference):
    gens    = argmax(logits, axis=-1)                     # [B,T]
    sel     = assoc_mask[gens]                            # [B,T,C]
    attnsum = einsum('btc,bct->bt', sel, attns)
    attnloss = mean(where(any(sel,-1), (1-attnsum)^2, 0))
    nll     = logsumexp(logits) - logits[target]
    xent    = sum((target!=0)*nll) / sum(target!=0)
    out     = xent + attnloss                             # f32 scalar

Sharding: data-parallel on the flattened (B*T)=4096 rows, 512 rows per
core across 8 cores.  Each core streams its 64 MB logits shard once
(memory-bound regime): per chunk one DVE grouped reduce_max and one
ScalarE Exp pass with accum_out (free-dim sum of exp written straight
into the output tile).  The exact first-occurrence argmax is recovered
per row-group from the group maxes (max + max_index), an indirect
re-gather of the winning group from DRAM, and a second max/max_index.
The target logit is indirect-gathered on device.

v3 notes (after trace analysis of v1/v2):
  - DVE grouped reduce has a ~115ns per-group overhead, so rg0-2 use
    1000-wide groups (G=32); rg3 (the last-resolved row group) keeps
    500-wide groups (G=64) so its tail gather + find is cheap.
  - rg3 streams as [4000x6, 2000x3, 1000, 500, 500] so no single
    reduce exceeds the DMA cadence and the final reduce on the tail
    critical path is ~0.8us instead of ~10us.
  - part1(rg) issues immediately after rg's last chunk; part2(rg)
    waits until two chunks into the next row group: the winning-group
    indirect gather takes 11-16us under full streaming load, and the
    in-order DVE queue must never stall on it (v2 regression: a
    stalled DVE delays tile-slot reuse which gaps the HBM stream).
  - part2 reuses part1's m8 (the global row max IS the max within the
    winning group), skipping a second max pass over the gathered group.
  - The attention-loss term is computed host-side from the shipped
    gens indices; this removes the assoc-mask gathers and the attn
    dot from the device tail.
  - Single output tensor, gens stored as f32 (< 2^24 exact): one
    early ship (rg0-2 partials) and one tail ship (rg3 partials).

Per-core output: [128, 33] partials ([13 rg0-2 exp-sum | 4 target
logit | 3 gens | 12 rg3 exp-sum | 1 gens] columns); the host does the
final log + scalar reduction and the tiny attention-loss dot.
"""

import numpy as np

import concourse.bass as bass
from concourse import bacc, mybir
from concourse.bass_utils import run_bass_kernel_spmd
from concourse.tile import TileContext

# Problem shape (hardcoded; kernel.py must be self-contained).
B, T, V, C = 4, 1024, 32000, 64
NCORES = 8
P = 128                    # SBUF partitions
R = (B * T) // NCORES      # rows (positions) per core = 512
RG = R // P                # row-groups per core = 4
CH = 8000                  # max chunk free size (tile width)

# per-row-group chunk column spans; rg0's first chunk is split so the
# first DVE reduce starts early, rg3 is finely chunked so no reduce
# exceeds the DMA cadence and the final one is tiny.
CHUNK_SPANS = [
    [(0, 2000), (2000, 10000), (10000, 16000), (16000, 24000), (24000, 32000)],
    [(0, 8000), (8000, 16000), (16000, 24000), (24000, 32000)],
    [(0, 8000), (8000, 16000), (16000, 24000), (24000, 32000)],
    [(0, 4000), (4000, 8000), (8000, 12000), (12000, 16000),
     (16000, 20000), (20000, 24000), (24000, 26000), (26000, 28000),
     (28000, 29500), (29500, 31000), (31000, 31500), (31500, 32000)],
]
NCH = [len(sp) for sp in CHUNK_SPANS]          # 5,4,4,12
# per-row-group argmax group width (chunk spans must be multiples)
SGRP = [2000, 2000, 2000, 500]
GRPS = [V // s for s in SGRP]                  # groups per row

# output column layout:
#   [ss0(5) | ss1(4) | ss2(4) | tv(4) | g0 g1 g2 | ss3(12) | g3]
SS_COL = [0, 5, 9, 20]
OUT_TV = 13
GCOL = [17, 18, 19, 32]
EARLY_W = 20
OUT_W = 33

_DT = mybir.dt


def build_nc() -> bass.Bass:
    """Build the per-core Bass program (SPMD: identical on all cores)."""
    nc = bacc.Bacc(
        "TRN2", target_bir_lowering=False, debug=False, num_devices=NCORES
    )

    lg = nc.dram_tensor("lg", [R * V], _DT.float32, kind="ExternalInput")
    tofs = nc.dram_tensor("tofs", [P, RG], _DT.uint32, kind="ExternalInput")
    out = nc.dram_tensor("out", [P, OUT_W], _DT.float32, kind="ExternalOutput")

    # Views of the logits shard.
    lg2d = lg[:].rearrange("(r v) -> r v", v=V)       # [512, 32000]
    lg_grp = [lg[:].rearrange("(n s) -> n s", s=s) for s in SGRP]
    lg_e = lg[:].rearrange("(n o) -> n o", o=1)       # [512*32000, 1]

    fp32 = _DT.float32
    u32 = _DT.uint32
    AX = mybir.AxisListType.X
    OP = mybir.AluOpType

    with TileContext(nc) as tc:
        with (
            tc.tile_pool(name="chunks", bufs=5) as chunks,
            tc.tile_pool(name="expo", bufs=1) as expo,
            tc.tile_pool(name="small", bufs=2) as small,
            tc.tile_pool(name="grppool", bufs=1) as grppool,
            tc.tile_pool(name="consts", bufs=1) as consts,
        ):
            # ---- preamble: constants + everything independent of logits ----
            pi = consts.tile([P, 1], _DT.int32)
            nc.gpsimd.iota(pi[:], [[1, 1]], base=0, channel_multiplier=1)
            pf = consts.tile([P, 1], fp32)
            nc.gpsimd.tensor_copy(out=pf[:], in_=pi[:])
            rb = consts.tile([P, RG], fp32)   # (rg*128+p)*G_rg per col
            for rg in range(RG):
                nc.gpsimd.tensor_scalar(
                    out=rb[:, rg:rg + 1], in0=pf[:],
                    scalar1=float(GRPS[rg]), scalar2=float(rg * P * GRPS[rg]),
                    op0=OP.mult, op1=OP.add,
                )

            # preamble loads go on the scalar HWDGE ring so chunk(0,0) is
            # the first transfer on the sync ring
            tofs_sb = consts.tile([P, RG], u32)
            nc.scalar.dma_start(out=tofs_sb[:], in_=tofs[:])

            out_sb = consts.tile([P, OUT_W], fp32)

            # target-logit gathers: independent of everything downstream
            for rg in range(RG):
                nc.gpsimd.indirect_dma_start(
                    out=out_sb[:, OUT_TV + rg:OUT_TV + rg + 1],
                    out_offset=None,
                    in_=lg_e,
                    in_offset=bass.IndirectOffsetOnAxis(
                        ap=tofs_sb[:, rg:rg + 1], axis=0
                    ),
                )

            # ---- streaming + interleaved resolution ----
            mc_tiles = {}
            st = {}  # per-rg resolution state (small tiles)

            from concourse.tile import add_dep_helper

            red = {}  # (rg, k) -> reduce instruction, for ordering edges

            def after(binst, dep, why):
                # Ordering-only edge: binst must not be scheduled before dep.
                add_dep_helper(binst.ins, dep.ins, sync=False, reason=why)

            def chunk(rg, k):
                lo, hi = CHUNK_SPANS[rg][k]
                w = hi - lo
                s = SGRP[rg]
                t = chunks.tile([P, CH], fp32, name=f"t_{rg}_{k}", tag="t")
                # alternate the two HWDGE rings (sync / scalar sequencers)
                glob = sum(NCH[:rg]) + k
                dma_eng = nc.sync if glob % 2 == 0 else nc.scalar
                dma_eng.dma_start(
                    out=t[:, :w],
                    in_=lg2d[rg * P:(rg + 1) * P, lo:hi],
                )
                t3 = t[:, :w].rearrange("p (g s) -> p g s", s=s)
                red[(rg, k)] = nc.vector.tensor_reduce(
                    out=mc_tiles[rg][:, lo // s:hi // s],
                    in_=t3,
                    axis=AX,
                    op=OP.max,
                )
                sscol = SS_COL[rg] + k
                eo = expo.tile([P, CH], fp32, name=f"eo_{rg}_{k}", tag="eo")
                nc.scalar.activation(
                    out=eo[:, :w],
                    in_=t[:, :w],
                    func=mybir.ActivationFunctionType.Exp,
                    accum_out=out_sb[:, sscol:sscol + 1],
                )

            def part1(rg, dep=None):
                # global max + winning group; issue the group re-gather.
                # Small ops run on GpSimd for rg0-2 (saves DVE time) but on
                # Vector for rg3, whose chain is the latency-critical tail.
                eng = nc.vector if rg == RG - 1 else nc.gpsimd
                mc = mc_tiles[rg]
                m8 = small.tile([P, 8], fp32, name=f"m8_{rg}", tag="m8")
                i = nc.vector.max(out=m8[:], in_=mc[:])
                if dep is not None:
                    after(i, dep, f"part1({rg}) placement")
                g8 = small.tile([P, 8], u32, name=f"g8_{rg}", tag="g8")
                nc.vector.max_index(g8[:], m8[:], mc[:])
                g8f = small.tile([P, 1], fp32, name=f"g8f_{rg}", tag="g8f")
                eng.tensor_copy(out=g8f[:], in_=g8[:, 0:1])
                gidxf = small.tile([P, 1], fp32, name=f"gxf_{rg}", tag="gxf")
                eng.tensor_tensor(
                    out=gidxf[:], in0=rb[:, rg:rg + 1], in1=g8f[:], op=OP.add
                )
                gidx = small.tile([P, 1], u32, name=f"gx_{rg}", tag="gx")
                eng.tensor_copy(out=gidx[:], in_=gidxf[:])
                grp = grppool.tile(
                    [P, SGRP[rg]], fp32, name=f"grp_{rg}", tag=f"grp{SGRP[rg]}"
                )
                gather = nc.gpsimd.indirect_dma_start(
                    out=grp[:],
                    out_offset=None,
                    in_=lg_grp[rg],
                    in_offset=bass.IndirectOffsetOnAxis(ap=gidx[:, :1], axis=0),
                )
                st[rg] = {"g8f": g8f, "grp": grp, "m8": m8, "gather": gather}

            def part2(rg, dep=None):
                # index within the winning group -> gens (f32 out column).
                # part1's m8[:,0] (global row max) IS the max within the
                # winning group, so no second max pass is needed.
                eng = nc.vector if rg == RG - 1 else nc.gpsimd
                grp = st[rg]["grp"]
                j8 = small.tile([P, 8], u32, name=f"j8_{rg}", tag="j8")
                i = nc.vector.max_index(j8[:], st[rg]["m8"][:], grp[:])
                if dep is not None:
                    after(i, dep, f"part2({rg}) placement")
                j8f = small.tile([P, 1], fp32, name=f"j8f_{rg}", tag="j8f")
                eng.tensor_copy(out=j8f[:], in_=j8[:, 0:1])
                gensf = small.tile([P, 1], fp32, name=f"gf_{rg}", tag="gf")
                eng.tensor_scalar(
                    out=gensf[:], in0=st[rg]["g8f"][:],
                    scalar1=float(SGRP[rg]), scalar2=None, op0=OP.mult,
                )
                eng.tensor_tensor(
                    out=out_sb[:, GCOL[rg]:GCOL[rg] + 1],
                    in0=gensf[:], in1=j8f[:], op=OP.add,
                )

            def alloc_mc(rg):
                mc_tiles[rg] = small.tile(
                    [P, GRPS[rg]], fp32, name=f"mc_{rg}", tag=f"mc{GRPS[rg]}"
                )

            # Schedule: part1(rg) fires as soon as rg's group maxes are
            # complete (gather gets a full ~2-chunk head start); part2(rg)
            # is deferred two chunks into the next row group so the DVE
            # queue never waits on an in-flight gather.
            for rg in range(RG):
                alloc_mc(rg)
                # rg3's chunks are half-width, so defer its predecessor's
                # part2 one extra chunk to keep ~20us+ of gather lead time
                p2k = 3 if rg == RG - 1 else 2
                for k in range(NCH[rg]):
                    chunk(rg, k)
                    if rg > 0 and k == p2k:
                        part2(rg - 1, dep=red[(rg, k)])
                part1(rg, dep=red[(rg, NCH[rg] - 1)])

            # rg0-2 results ship as soon as they're complete (ssum, tv,
            # gens cols); only rg3's partials wait for the tail.
            nc.sync.dma_start(out=out[:, 0:EARLY_W], in_=out_sb[:, 0:EARLY_W])

            # Tail: rg3's within-group resolution.
            part2(RG - 1)
            nc.sync.dma_start(
                out=out[:, EARLY_W:OUT_W], in_=out_sb[:, EARLY_W:OUT_W]
            )

    nc.compile()
    return nc


_NC_CACHE: list = []


def _get_nc() -> bass.Bass:
    if not _NC_CACHE:
        _NC_CACHE.append(build_nc())
    return _NC_CACHE[0]


def make_in_maps(logits, targets):
    """Host-side sharding: per-core input dicts."""
    logits = np.asarray(logits, dtype=np.float32)
    targets = np.asarray(targets).astype(np.int64)

    lg_all = logits.reshape(B * T, V)
    tflat = targets.reshape(B * T)

    in_maps = []
    for c in range(NCORES):
        r0 = c * R
        lg_c = np.ascontiguousarray(lg_all[r0:r0 + R]).reshape(R * V)
        tgt_c = tflat[r0:r0 + R]
        # flat element offset of the target logit within this core's shard,
        # laid out [partition, row-group]: row r = rg*128 + p
        tofs_c = (np.arange(R, dtype=np.int64) * V + tgt_c).reshape(RG, P).T
        in_maps.append({
            "lg": lg_c,
            "tofs": np.ascontiguousarray(tofs_c).astype(np.uint32),
        })
    return in_maps


def combine_results(results, targets, attns, assoc_mask):
    """Host-side reduction of the per-core [128, OUT_W] partials."""
    targets = np.asarray(targets).astype(np.int64)
    tflat = targets.reshape(B * T)
    amask = np.asarray(assoc_mask).astype(bool)           # [V, C]
    attns = np.asarray(attns, dtype=np.float64)           # [B, C, T]

    wnll = 0.0
    wsum = 0.0
    gens_all = np.empty(B * T, dtype=np.int64)
    for c in range(NCORES):
        o = np.asarray(results[c]["out"], dtype=np.float64)  # [128, OUT_W]
        ssum = np.stack(
            [
                o[:, SS_COL[rg]:SS_COL[rg] + NCH[rg]].sum(axis=1)
                for rg in range(RG)
            ],
            axis=1,
        )
        lse = np.log(ssum)                     # [128, RG]
        tv = o[:, OUT_TV:OUT_TV + RG]
        nll = (lse - tv).T.reshape(R)          # row r = rg*128 + p
        tgt_c = tflat[c * R:(c + 1) * R]
        w = (tgt_c != 0).astype(np.float64)
        wnll += float((w * nll).sum())
        wsum += float(w.sum())
        gens_all[c * R:(c + 1) * R] = (
            o[:, GCOL].astype(np.int64).T.reshape(R)
        )

    # attention-loss term from the device-computed argmax indices
    sel = amask[gens_all]                                  # [B*T, C] bool
    att = np.ascontiguousarray(attns.transpose(0, 2, 1)).reshape(B * T, C)
    attnsum = (sel * att).sum(axis=1)
    has = sel.any(axis=1)
    attnloss = float(np.where(has, (1.0 - attnsum) ** 2, 0.0).mean())

    loss = wnll / wsum + attnloss
    return np.array(loss, dtype=np.float32)


def kernel(**inputs) -> np.ndarray:
    in_maps = make_in_maps(inputs["logits"], inputs["targets"])
    nc = _get_nc()
    res = run_bass_kernel_spmd(nc, in_maps, core_ids=list(range(NCORES))).results
    return combine_results(
        res, inputs["targets"], inputs["attns"], inputs["assoc_mask"]
    )
